# revision 70
# baseline (speedup 1.0000x reference)
"""Trainium2 Bass kernel for nn_AV_MiltiHeadAttention.

Strategy: data-parallel over B across 8 NeuronCores (1 batch element per core).
Per-core everything is kept in a "transposed" (feature-on-partitions) layout so
no on-chip transposes are needed:

  audiaT/lipT/W*T are marshalled on host (transpose + bf16 cast = input
  layout prep for the SPMD shards), all matmul contractions then have their
  contraction dim on SBUF partitions naturally.

  S.T[m,n] = sum_d kTz[d,m] qT[d,n]         (zero-padded per head: K=128, so
                                             every matmul shares one PE tiling
                                             mode -> no array-drain switches)
  E = exp(S.T * scale)                       (no max subtraction; |S*scale|<2)
  raw = [ones64|k_h].T @ E (parity-flipped) -> one matmul yields BOTH the
        unnormalized attn_qk.T rows AND the softmax denominator r broadcast
        across a 64-row block, fp32 in PSUM; a full-tile in-place DVE recip
        (base-0 constrained) gives 1/r, and the normalize tensor_tensor is
        fully partition-aligned.  No [1,N] lane-starved recip chains, no
        separate broadcast matmuls.
  y = attn_qk.T * lip_v.T ; expY = exp(y)
  1/s via ONE matmul against the bcHH block stationary ((d<64)==(j<64)):
  per-head head_dim sums of expY land already broadcast on all 128
  partitions (softmax over head_dim), fp32, then in-place recip.
  z.T = attn_qk.T * expY * (1/s)
  av_attn = 0.5 + 0.5*tanh(qs.T@qs / temp)   (SE layer; tanh lives in the
                                             same ACT table set as exp -> no
                                             mid-kernel ACT_TABLE_LOADs)
  Wc = W_proj.T @ av_attn ; row = b_proj @ av_attn   (proj/out fusion)
  out = z @ Wc + row                          (bias + cb<3 partials
                                             accumulate early, straddling
                                             the front(3)/back(3) stall
                                             windows; the partial re-enters
                                             the final psum via a K=128
                                             identity matmul so the tail is
                                             one evac + DMA per chunk)

All matmuls bf16 x bf16 -> fp32 PSUM. Softmax normalization in fp32.
"""

import os
import numpy as np
import ml_dtypes

DEBUG_DUMP = os.environ.get("KDBG", "0") == "1"

B, N, C = 8, 1024, 512
H, HD = 8, 64
CB = C // 128          # 4 chunks of the feature dim
MB = N // 128          # 8 chunks of the token dim
NH = N // 512          # 2 halves of the token dim (matmul free dim = 512)
SCALE = HD ** -0.5
TEMP = C ** 0.5

_CACHED = {}


def build_nc():
    import concourse.bass as bass
    import concourse.tile as tile
    import concourse.mybir as mybir
    from concourse import bacc
    from contextlib import ExitStack

    f32 = mybir.dt.float32
    bf16 = mybir.dt.bfloat16
    AF = mybir.ActivationFunctionType
    MUL = mybir.AluOpType.mult
    ADD = mybir.AluOpType.add

    nc = bacc.Bacc("TRN2", target_bir_lowering=False, debug=False, num_devices=B)

    d_audiaT = nc.dram_tensor("audiaT", [C, N], bf16, kind="ExternalInput")
    d_lipT = nc.dram_tensor("lipT", [C, N], bf16, kind="ExternalInput")
    d_WqkT = nc.dram_tensor("WqkT", [C, 2 * C], bf16, kind="ExternalInput")
    d_WlipT = nc.dram_tensor("WlipT", [C, C], bf16, kind="ExternalInput")
    d_WseT = nc.dram_tensor("WseT", [C, C], bf16, kind="ExternalInput")
    d_WprojN = nc.dram_tensor("WprojN", [C, C], bf16, kind="ExternalInput")
    d_bprojP = nc.dram_tensor("bprojP", [128, CB, HD + 1], bf16, kind="ExternalInput")
    d_ident = nc.dram_tensor("ident", [128, 128], bf16, kind="ExternalInput")
    d_bcHH = nc.dram_tensor("bcHH", [128, 128], bf16, kind="ExternalInput")
    d_onesK = nc.dram_tensor("onesK", [HD + 1, 128], bf16, kind="ExternalInput")
    d_out = nc.dram_tensor("out", [N, C], bf16, kind="ExternalOutput")
    dbg = {}
    if DEBUG_DUMP:
        for nm, shp, dt in [
            ("dbg_qT", [128, CB, N], bf16), ("dbg_kTz", [128, CB, 2, N], bf16),
            ("dbg_kaug", [128, MB, H * 2 * HD], bf16),
            ("dbg_lipv", [128, CB, N], bf16), ("dbg_qs", [128, MB, C], bf16),
            ("dbg_E0", [128, MB, 2 * N], bf16),
            ("dbg_rawsb0", [HD, N], bf16), ("dbg_attn0", [128, N], bf16),
            ("dbg_r00", [1, N], f32),
            ("dbg_y0", [128, N], bf16), ("dbg_expY0", [128, N], bf16),
            ("dbg_sblo0", [HD, N], f32), ("dbg_sbhi0", [HD, N], f32),
            ("dbg_zT", [128, CB, N], bf16), ("dbg_avattn", [128, CB, C], bf16),
            ("dbg_wc", [128, CB, C], bf16),
        ]:
            dbg[nm] = nc.dram_tensor(nm, shp, dt, kind="ExternalOutput")

    with tile.TileContext(nc) as tc, ExitStack() as ctx:
        persist = ctx.enter_context(tc.tile_pool(name="persist", bufs=1))
        # psA: 2x [128,1024] fp32 (2 banks each) for the S/qkT/partial/final
        # streams; psB: 4x [128,512] (1 bank each) -- fine-grained ring for
        # the raw/normalize chains so a held tile blocks less of the ring.
        psA = ctx.enter_context(tc.tile_pool(name="psA", bufs=2, space="PSUM"))
        psB = ctx.enter_context(tc.tile_pool(name="psB", bufs=4, space="PSUM"))

        # ---- persistent SBUF tensors ----
        qT = persist.tile([128, CB, N], bf16, tag="qT")        # q.T  [(h,d), n]
        # k.T zero-padded per head so the S matmuls run K=128 (no PE
        # tiling-mode switches): chunk 0 = [k_h_lo; 0], chunk 1 = [0; k_h_hi]
        # (measured: 64-row tiling loses ~18us to mode-switch drains because
        # the scheduler interleaves S with 128-mode matmuls, and the pairs
        # never co-stream -- the psA ring is ACT-gated.  Zeros via the idle
        # GpSimd engine, off the DVE.)
        kTz = persist.tile([128, CB, 2, N], bf16, tag="kTz")
        # k natural, one 128-col block per head: even heads [ones64 | k_h],
        # odd heads [k_h | ones64].  The raw matmul then emits BOTH the
        # unnormalized attn rows AND the softmax denominator r broadcast
        # across a 64-row block (in fp32, straight from the PE) -- and the
        # parity flip makes every downstream evac/recip/normalize-TT
        # partition-aligned with the packed raw2/attn_sb layout.
        k_aug = persist.tile([128, MB, H * 2 * HD], bf16, tag="k_aug")
        lip_vT = persist.tile([128, CB, N], bf16, tag="lip_vT")
        qs = persist.tile([128, MB, C], bf16, tag="qs")        # qs natural [n, c]
        WprojN_sb = persist.tile([128, CB, C], bf16, tag="WprojN_sb")
        Wc_sb = persist.tile([128, CB, C], bf16, tag="Wc_sb")
        bprojP_sb = persist.tile([128, CB, HD + 1], bf16, tag="bprojP_sb")
        ident_sb = persist.tile([128, 128], bf16, tag="ident_sb")
        # block stationary W[d,j] = ((d<64)==(j<64)): one matmul computes the
        # per-head head_dim sums of expY already BROADCAST across all 128
        # output partitions, in fp32 -- replaces the old sel-matmul ->
        # psum->sbuf s2 copies -> selC-broadcast-matmul chain entirely.
        bcHH_sb = persist.tile([128, 128], bf16, tag="bcHH_sb")
        zT = persist.tile([128, CB, N], bf16, tag="zT")
        av_attn = persist.tile([128, CB, C], bf16, tag="av_attn")
        # bias row for the output GEMM, as K=65 rhs (rows 1:64 zeroed so the
        # onesK zero-rows multiply clean values, never junk NaNs)
        row_bf = persist.tile([HD + 1, C], bf16, tag="row_bf")
        nc.gpsimd.memset(row_bf[:], 0.0)
        # (NOTE: a PE warm-up spin of dependency-free matmuls here was tried
        # to beat the HAM cold-clock and measured consistently ~25us SLOWER
        # overall -- do not reintroduce.)
        # onesK row 0 -> all cols (K=65 rounds to row-size 128: no PE
        # tiling-mode switch): folds the bias row into the partial-output
        # psum chains via one matmul.
        onesK = persist.tile([HD + 1, 128], bf16, tag="onesK")
        nc.gpsimd.dma_start(onesK[:], d_onesK[:])
        # kTz zero-padding memsets ride the early, otherwise-idle gpsimd
        # queue (the S(0) matmuls that read them start at ~17us)
        nc.gpsimd.memset(kTz[64:128, :, 0, :], 0.0)
        nc.gpsimd.memset(kTz[0:64, :, 1, :], 0.0)

        attn = ctx.enter_context(tc.tile_pool(name="attn", bufs=1))
        early_ctx = tc.tile_pool(name="early", bufs=1)
        early = early_ctx.__enter__()
        audiaT_sb = early.tile([128, CB, N], bf16, tag="audiaT_sb")
        lipT_sb = early.tile([128, CB, N], bf16, tag="lipT_sb")
        WqkT_sb = early.tile([128, CB, 2 * C], bf16, tag="WqkT_sb")
        WlipT_sb = early.tile([128, CB, C], bf16, tag="WlipT_sb")
        WseT_sb = early.tile([128, CB, C], bf16, tag="WseT_sb")

        # input DMAs ordered by first use and spread across engine DMA rings:
        # qkT needs WqkT(q half) [sync ring] + audiaT [scalar ring] first;
        # the k half [vector ring] feeds emit_kaug; lip/se/proj follow.
        d_WqkT_r = d_WqkT[:].rearrange("(cb p) c -> cb p c", p=128)
        d_audiaT_r = d_audiaT[:].rearrange("(cb p) n -> cb p n", p=128)
        # (NOTE: splitting the leading WqkT/audiaT chunks into smaller first
        # transfers was tried and measured ~4us SLOWER -- the extra ~650ns
        # descriptor submits delay the following chunks more than the
        # earlier first-semaphore gains.)
        for cb in range(CB):
            nc.sync.dma_start(WqkT_sb[:, cb, 0:C], d_WqkT_r[cb][:, 0:C])
            nc.scalar.dma_start(audiaT_sb[:, cb], d_audiaT_r[cb])
        for cb in range(CB):
            nc.sync.dma_start(WqkT_sb[:, cb, C:2 * C], d_WqkT_r[cb][:, C:2 * C])
        nc.scalar.dma_start(WlipT_sb[:], d_WlipT[:].rearrange("(cb p) c -> p cb c", p=128))
        nc.scalar.dma_start(lipT_sb[:], d_lipT[:].rearrange("(cb p) n -> p cb n", p=128))
        nc.sync.dma_start(bcHH_sb[:], d_bcHH[:])
        nc.sync.dma_start(WseT_sb[:], d_WseT[:].rearrange("(cb p) c -> p cb c", p=128))
        nc.scalar.dma_start(WprojN_sb[:], d_WprojN[:].rearrange("(cb p) c -> p cb c", p=128))
        nc.scalar.dma_start(bprojP_sb[:], d_bprojP[:])
        nc.gpsimd.dma_start(ident_sb[:], d_ident[:])



        # ---- P1 pieces (emitted interleaved with attention below) ----
        def emit_qkT():
            # qT / kTz: out [oc:128, n] ; lhsT = WqkT block, rhs = audiaT
            for dst, oc0 in ((qT, 0), (kTz, C)):
                for ocb in range(CB):
                    ps = psA.tile([128, N], f32, tag="psA")
                    for cb in range(CB):
                        for nh in range(NH):
                            nc.tensor.matmul(
                                ps[:, nh * 512:(nh + 1) * 512],
                                WqkT_sb[:, cb, oc0 + ocb * 128: oc0 + (ocb + 1) * 128],
                                audiaT_sb[:, cb, nh * 512:(nh + 1) * 512],
                                start=(cb == 0), stop=(cb == CB - 1),
                            )
                    if dst is qT:
                        nc.vector.tensor_copy(out=dst[:, ocb, :], in_=ps[:])
                    else:
                        nc.vector.tensor_copy(out=kTz[0:64, ocb, 0, :], in_=ps[0:64, :])
                        # DVE, not ACT: the E0/E1 exps own the ACT queue
                        # here and pace the whole stage
                        nc.vector.tensor_copy(out=kTz[64:128, ocb, 1, :],
                                              in_=ps[64:128, :])

        def emit_kaug():
            # k natural into the per-parity block slots; ones blocks memset
            # once up front (even heads: block 0 = ones; odd: block 1)
            kv = k_aug[:].rearrange(
                "p m (hp par two d) -> p m hp par two d", par=2, two=2, d=HD
            )
            nc.gpsimd.memset(kv[:, :, :, 0, 0, :], 1.0)
            nc.gpsimd.memset(kv[:, :, :, 1, 1, :], 1.0)
            for mb in range(MB):
                ps = psB.tile([128, 512], f32, tag="psB")
                for cb in range(CB):
                    nc.tensor.matmul(
                        ps[:],
                        audiaT_sb[:, cb, mb * 128:(mb + 1) * 128],
                        WqkT_sb[:, cb, C:2 * C],
                        start=(cb == 0), stop=(cb == CB - 1),
                    )
                psv = ps[:].rearrange("p (hp par d) -> p hp par d", par=2, d=HD)
                kvm = k_aug[:, mb].rearrange(
                    "p (hp par two d) -> p hp par two d", par=2, two=2, d=HD
                )
                nc.vector.tensor_copy(out=kvm[:, :, 0, 1, :], in_=psv[:, :, 0, :])
                nc.vector.tensor_copy(out=kvm[:, :, 1, 0, :], in_=psv[:, :, 1, :])

        def emit_lipv():
            for ocb in range(CB):
                pss = [psB.tile([128, 512], f32, tag="psB", name=f"lipv_ps{ocb}_{i}")
                       for i in range(NH)]
                for cb in range(CB):
                    for nh in range(NH):
                        nc.tensor.matmul(
                            pss[nh][:],
                            WlipT_sb[:, cb, ocb * 128:(ocb + 1) * 128],
                            lipT_sb[:, cb, nh * 512:(nh + 1) * 512],
                            start=(cb == 0), stop=(cb == CB - 1),
                        )
                for nh in range(NH):
                    nc.vector.tensor_copy(
                        out=lip_vT[:, ocb, nh * 512:(nh + 1) * 512], in_=pss[nh][:]
                    )

        def emit_qs():
            # qs natural [n, c] (needed on both sides of the SE bmm)
            for nb in range(MB):
                ps = psB.tile([128, 512], f32, tag="psB")
                for cb in range(CB):
                    nc.tensor.matmul(
                        ps[:],
                        audiaT_sb[:, cb, nb * 128:(nb + 1) * 128],
                        WseT_sb[:, cb, :],
                        start=(cb == 0), stop=(cb == CB - 1),
                    )
                nc.vector.tensor_copy(out=qs[:, nb, :], in_=ps[:])

        # ---- attention + SE ----
        def emit_se():
            # x[c,d] = sum_n qs[n,c] qs[n,d];  av_attn = sigmoid(2x/temp)
            # computed as 0.5 + 0.5*tanh(x/temp): tanh lives in the same ACT
            # table set as exp, so no mid-kernel ACT_TABLE_LOAD thrash.
            # The psum is evacuated to SBUF by a fast DVE cast FIRST: the
            # tanh queues on ACT behind all the E3 exps, and letting it hold
            # the psB tile hostage blocked front(3)'s raw allocations for
            # ~7us (the longest drain chain to the kernel end).
            for cb in range(CB):
                ps = psB.tile([128, 512], f32, tag="psB")
                for nb in range(MB):
                    nc.tensor.matmul(
                        ps[:],
                        qs[:, nb, cb * 128:(cb + 1) * 128],
                        qs[:, nb, :],
                        start=(nb == 0), stop=(nb == MB - 1),
                    )
                st = attn.tile([128, C], bf16, tag="sest", bufs=2, name=f"sest{cb}")
                nc.vector.tensor_copy(out=st[:], in_=ps[:])
                th = attn.tile([128, C], bf16, tag="th", bufs=2, name=f"th{cb}")
                nc.scalar.activation(th[:], st[:], AF.Tanh, scale=1.0 / TEMP)
                nc.vector.tensor_scalar(av_attn[:, cb, :], th[:], 0.5, 0.5, MUL, ADD)

        def emit_wc():
            # Wc = W_proj.T @ av_attn  (fold the proj GEMM into the output GEMM)
            for ccb in range(CB):
                ps = psA.tile([128, N], f32, tag="psA")
                for eb in range(CB):
                    nc.tensor.matmul(
                        ps[:, 0:512],
                        WprojN_sb[:, eb, ccb * 128:(ccb + 1) * 128],
                        av_attn[:, eb, :],
                        start=(eb == 0), stop=(eb == CB - 1),
                    )
                # DVE, not ACT: the E3 exps own the ACT queue here
                nc.vector.tensor_copy(out=Wc_sb[:, ccb, :], in_=ps[:, 0:512])
            # row = b_proj @ av_attn (M=65 zero-padded stationary: no tiling
            # switch); prefolded into the partial-output psum chains via a
            # K=65 onesK matmul.
            rp = psB.tile([128, 512], f32, tag="psB")
            for cb in range(CB):
                nc.tensor.matmul(
                    rp[0:HD + 1, :],
                    bprojP_sb[:, cb, :],
                    av_attn[:, cb, :],
                    start=(cb == 0), stop=(cb == CB - 1),
                )
            nc.vector.tensor_copy(out=row_bf[0:1, :], in_=rp[0:1, :])
            if DEBUG_DUMP:
                nc.sync.dma_start(dbg["dbg_wc"][:], Wc_sb[:])

        def emit_S(p):
            # E = exp(S.T * scale) for heads (2p, 2p+1)
            E = attn.tile([128, MB, 2 * N], bf16, tag="E", bufs=2, name=f"E{p}")
            for mb in range(MB):
                for hh in range(2):
                    ps = psA.tile([128, N], f32, tag="psA")
                    for nh in range(NH):
                        nc.tensor.matmul(
                            ps[:, nh * 512:(nh + 1) * 512],
                            kTz[:, p, hh, mb * 128:(mb + 1) * 128],
                            qT[:, p, nh * 512:(nh + 1) * 512],
                            start=True, stop=True,
                        )
                    nc.scalar.activation(
                        E[:, mb, hh * N:(hh + 1) * N], ps[:], AF.Exp, scale=SCALE
                    )
            return E

        def emit_qk_front(p, E):
            # raw = [k_h|1].T @ E_h for BOTH heads back-to-back (keeps the PE
            # dense so HAM stays at full clock), then the first-softmax
            # normalize chain through y/expY.  The s-path + z live in
            # emit_qk_back so PE filler work can be emitted between them
            # (the PE queue is in-order: a stalled s_ps matmul would block
            # every filler emitted after it).
            attn_sb = attn.tile([128, N], bf16, tag="attn_sb", bufs=2, name=f"attn_sb{p}")
            raw2 = attn.tile([128, N], bf16, tag="raw_sb", bufs=2, name=f"raw2_{p}")
            rawps = {}
            for hh in range(2):
                h = 2 * p + hh
                raws = [psB.tile([128, 512], f32, tag="psB", name=f"raw{p}_{hh}_{i}")
                        for i in range(NH)]
                rawps[hh] = raws
                for mb in range(MB):
                    for nh in range(NH):
                        nc.tensor.matmul(
                            raws[nh][:],
                            k_aug[:, mb, h * 128:(h + 1) * 128],
                            E[:, mb, hh * N + nh * 512: hh * N + (nh + 1) * 512],
                            start=(mb == 0), stop=(mb == MB - 1),
                        )
            # even head: psum rows 0:64 = r broadcast, rows 64:128 = unnorm
            # attn; odd head mirrored.  Normalize runs nh-outer so each
            # column half flows evac -> recip -> TT -> y -> expY as one
            # short chain (expY gates the s-path matmuls downstream).
            # Evacs: the attn rows move to the packed raw2 via partition-
            # shifted DVE copies; the full-tile in-place recip keeps the op
            # at partition base 0 (the DVE recip silently no-ops at base 64)
            # -- the attn rows it clobbers are already in raw2.  All evacs
            # stay off ACT: the exp streams own that queue and pace every
            # stage (an ACT-routed p==2 evac measured slower).
            y = attn.tile([128, N], bf16, tag="y", bufs=2, name=f"y{p}")
            expY = attn.tile([128, N], bf16, tag="expY", bufs=2, name=f"expY{p}")
            for nh in range(NH):
                sl = slice(nh * 512, (nh + 1) * 512)
                for hh in range(2):
                    a0 = 64 if hh == 0 else 0     # attn rows base in psum
                    nc.vector.tensor_copy(
                        out=raw2[hh * 64:hh * 64 + 64, sl],
                        in_=rawps[hh][nh][a0:a0 + 64, :],
                    )
                for hh in range(2):
                    nc.vector.reciprocal_approx_fast(
                        out=rawps[hh][nh][:], in_=rawps[hh][nh][:]
                    )
                for hh in range(2):
                    r0 = 0 if hh == 0 else 64
                    rows = slice(hh * 64, hh * 64 + 64)
                    nc.vector.tensor_tensor(
                        attn_sb[rows, sl], raw2[rows, sl],
                        rawps[hh][nh][r0:r0 + 64, :], MUL,
                    )
                # y gates expY -> the whole s-chain: keep it on the fast DVE
                # (a GpSimd TT takes 1.15us vs 0.33us and measured as the
                # largest late-phase PE gap when p==2 rode GpSimd).
                nc.vector.tensor_tensor(y[:, sl], attn_sb[:, sl],
                                        lip_vT[:, p, sl], MUL)
                nc.scalar.activation(expY[:, sl], y[:, sl], AF.Exp)
            if DEBUG_DUMP and p == 0:
                nc.sync.dma_start(dbg["dbg_rawsb0"][:], raw2[0:HD, :])
                nc.sync.dma_start(dbg["dbg_attn0"][:], attn_sb[:])
                nc.sync.dma_start(dbg["dbg_y0"][:], y[:])
                nc.sync.dma_start(dbg["dbg_expY0"][:], expY[:])
            return attn_sb, expY

        def emit_qk_back(p, attn_sb, expY):
            # ONE matmul against the bcHH block stationary computes the
            # per-head head_dim sums of expY already broadcast across all
            # 128 output partitions (rows 0:64 = s_lo, rows 64:128 = s_hi,
            # in fp32 -- no bf16 round-trip of s); an in-place full-width
            # reciprocal then yields 1/s.  The path runs at nh-half
            # granularity so zT's first half lands early - the output
            # finals for nb 0..3 only need columns 0:512.
            # For p==2 park the psums on psA - they hold their buffers to
            # the end of the chain and would starve the psB ring the se/wc
            # fillers need.  (p==3 must stay on psB: psA holds the output
            # partials.)
            if p == 2:
                sbt = psA.tile([128, N], f32, tag="psA")
                sbps = [sbt[:, 0:512], sbt[:, 512:1024]]
            else:
                sbps = [psB.tile([128, 512], f32, tag="psB", name=f"sb{p}_{i}")[:]
                        for i in range(NH)]
            u = attn.tile([128, N], bf16, tag="y", bufs=2, name=f"u{p}")
            for nh in range(NH):
                sl = slice(nh * 512, (nh + 1) * 512)
                nc.tensor.matmul(sbps[nh], bcHH_sb[:], expY[:, sl],
                                 start=True, stop=True)
                nc.vector.reciprocal_approx_fast(out=sbps[nh], in_=sbps[nh])
                # u is off the latency chain for p<2 (zT isn't needed until
                # the partials) -> GpSimd; p>=2 stays DVE (partials/finals
                # wait on zT 2/3, and a GpSimd TT costs 1.15us).
                if p <= 1:
                    nc.gpsimd.tensor_tensor(u[:, sl], attn_sb[:, sl], expY[:, sl], MUL)
                else:
                    nc.vector.tensor_tensor(u[:, sl], attn_sb[:, sl], expY[:, sl], MUL)
                nc.vector.tensor_tensor(zT[:, p, sl], u[:, sl], sbps[nh], MUL)

        # 2-deep software pipeline: S(p+1) overlaps qk(p); the independent
        # projection/SE matmuls are spread through the ACT-bound S windows
        # as PE filler.  se/wc fill qk(2)'s normalize window; the bias +
        # cb=0..2 partial output accumulation (evacuated to SBUF bf16 by the
        # idle ACT engine) fills qk(3)'s; only the cb=3 matmul + one add +
        # DMA remain after zT(3) lands.
        emit_qkT()
        E0 = emit_S(0)
        emit_kaug()
        E1 = emit_S(1)
        emit_lipv()
        fb0 = emit_qk_front(0, E0)
        if DEBUG_DUMP:
            nc.sync.dma_start(dbg["dbg_E0"][:], E0[:])
        E2 = emit_S(2)
        emit_qk_back(0, *fb0)
        emit_qs()
        early_ctx.__exit__(None, None, None)
        fb1 = emit_qk_front(1, E1)
        E3 = emit_S(3)
        emit_qk_back(1, *fb1)
        fb2 = emit_qk_front(2, E2)
        # NOTE: emitting se two stages earlier (to pull its tanh ahead of
        # the E3 exps) measured ~30us SLOWER -- its psB tiles hostage the
        # mid-stage ring.  Keep se/wc here.
        emit_se()
        emit_wc()
        emit_qk_back(2, *fb2)
        # bias + cb=0..2 partial output accumulation for all 8 nb chunks;
        # two nb chunks pack into each [128,1024] psA tile, evacuated to
        # partial_sb so the psA ring keeps cycling.
        late = ctx.enter_context(tc.tile_pool(name="late", bufs=1))
        partial_sb = late.tile([128, MB, C], bf16, tag="partial_sb")

        def emit_partials(ilo, ihi):
            for i in range(ilo, ihi):
                ps = psA.tile([128, N], f32, tag="psA")
                for half in range(2):
                    nb = 2 * i + half
                    nc.tensor.matmul(
                        ps[:, half * 512:(half + 1) * 512],
                        onesK[:], row_bf[:],
                        start=True, stop=False,
                    )
                    for cb in range(CB - 1):
                        nc.tensor.matmul(
                            ps[:, half * 512:(half + 1) * 512],
                            zT[:, cb, nb * 128:(nb + 1) * 128],
                            Wc_sb[:, cb, :],
                            start=False, stop=(cb == CB - 2),
                        )
                # all evacs on DVE: the E3 exps own the ACT queue in this
                # window and pace the stage
                for half in range(2):
                    nc.vector.tensor_copy(
                        out=partial_sb[:, 2 * i + half, :],
                        in_=ps[:, half * 512:(half + 1) * 512],
                    )

        # partials straddle BOTH stall windows: ~2 i-slots of ready matmul
        # work sit in the PE queue while front(3)'s DVE normalize chain
        # runs (else a blocked bcHH(3) at the queue head costs ~4us), and
        # one more slot fills back(3)'s s-chain window before the finals.
        emit_partials(0, 1)
        fb3 = emit_qk_front(3, E3)
        emit_partials(1, 3)
        emit_qk_back(3, *fb3)
        # nb 6,7 (the tail-critical chunks) accumulate bias + cb0..2
        # directly in psB -- no partial_sb round-trip, no identity matmul:
        # the finals just add cb3 and evacuate.  (The open accumulation
        # group survives interleaved matmuls to other banks: has_written
        # bits are per-element per-bank.)
        direct = [psB.tile([128, 512], f32, tag="psB", name=f"direct{nb}")
                  for nb in (6, 7)]
        for j, nb in enumerate((6, 7)):
            nc.tensor.matmul(direct[j][:], onesK[:], row_bf[:],
                             start=True, stop=False)
            for cb in range(CB - 1):
                nc.tensor.matmul(
                    direct[j][:],
                    zT[:, cb, nb * 128:(nb + 1) * 128],
                    Wc_sb[:, cb, :],
                    start=False, stop=False,
                )
        if DEBUG_DUMP:
            nc.sync.dma_start(dbg["dbg_qT"][:], qT[:])
            nc.sync.dma_start(dbg["dbg_kTz"][:], kTz[:])
            nc.sync.dma_start(dbg["dbg_kaug"][:], k_aug[:])
            nc.sync.dma_start(dbg["dbg_lipv"][:], lip_vT[:])
            nc.sync.dma_start(dbg["dbg_qs"][:], qs[:])
            nc.sync.dma_start(dbg["dbg_zT"][:], zT[:])
            nc.sync.dma_start(dbg["dbg_avattn"][:], av_attn[:])

        # ---- output finals: out[nb] = partial_sb[nb] + z[:,3] @ Wc[3] ----
        # the partial re-enters through the PE (K=128 identity matmul
        # accumulating into the same psum), so the tail needs only one
        # psum->sbuf evacuation per [128,1024] (alternating ACT/DVE) + DMA.
        d_out_r = d_out[:].rearrange("(nb p) c -> nb p c", p=128)
        with tc.tile_pool(name="outp", bufs=3) as outp:
            for i in range(3):
                ps = psA.tile([128, N], f32, tag="psA")
                for half in range(2):
                    nb = 2 * i + half
                    sl = slice(half * 512, (half + 1) * 512)
                    nc.tensor.matmul(
                        ps[:, sl],
                        zT[:, CB - 1, nb * 128:(nb + 1) * 128],
                        Wc_sb[:, CB - 1, :],
                        start=True, stop=False,
                    )
                    nc.tensor.matmul(
                        ps[:, sl], ident_sb[:], partial_sb[:, 2 * i + half, :],
                        start=False, stop=True,
                    )
                # one whole-tile evac per i, alternating engines (a per-half
                # ACT||DVE split was tried and measured ~4us slower: the ACT
                # halves collide with the expY(3)/o6 tail stream)
                o2 = outp.tile([128, 2, C], bf16, tag="o_sb")
                if i % 2 == 0:
                    nc.scalar.activation(o2[:], ps[:], AF.Copy)
                else:
                    nc.vector.tensor_copy(out=o2[:], in_=ps[:])
                for half in range(2):
                    nb = 2 * i + half
                    # drain the stores over the scalar+sync DMA rings only:
                    # a gpsimd-ring store here costs a ~2.3us gpsimd pipe
                    # DRAIN in the postamble, after everything else is done
                    if nb % 2 == 0:
                        nc.scalar.dma_start(d_out_r[nb], o2[:, half])
                    else:
                        nc.sync.dma_start(d_out_r[nb], o2[:, half])
            # nb 6,7: close the direct psB accumulation with the cb3 matmul,
            # one evac each (ACT/DVE split), DMA on separate rings
            o6 = outp.tile([128, C], bf16, tag="o_sb", name="o6")
            o7 = outp.tile([128, C], bf16, tag="o_sb", name="o7")
            for j, nb in enumerate((6, 7)):
                nc.tensor.matmul(
                    direct[j][:],
                    zT[:, CB - 1, nb * 128:(nb + 1) * 128],
                    Wc_sb[:, CB - 1, :],
                    start=False, stop=True,
                )
            nc.scalar.activation(o6[:], direct[0][:], AF.Copy)
            nc.vector.tensor_copy(out=o7[:], in_=direct[1][:])
            nc.scalar.dma_start(d_out_r[6], o6[:])
            nc.sync.dma_start(d_out_r[7], o7[:])

    nc.compile()
    return nc


def _marshal(audia, lip, W_qkv, W_lip, W_proj, b_proj, W_se):
    bf16 = ml_dtypes.bfloat16
    WqkT = np.ascontiguousarray(W_qkv[:2 * C].T.astype(bf16))
    WlipT = np.ascontiguousarray(W_lip.T.astype(bf16))
    WseT = np.ascontiguousarray(W_se.T.astype(bf16))
    WprojN = np.ascontiguousarray(W_proj.astype(bf16))
    bprojP = np.zeros((128, CB, HD + 1), bf16)
    bprojP[:, :, 0] = np.asarray(b_proj, np.float32).reshape(CB, 128).T.astype(bf16)
    ident = np.eye(128, dtype=np.float32).astype(bf16)
    bcHH = np.zeros((128, 128), bf16)
    bcHH[0:64, 0:64] = 1
    bcHH[64:128, 64:128] = 1
    onesK = np.zeros((HD + 1, 128), bf16)
    onesK[0, :] = 1
    in_maps = []
    for b in range(B):
        in_maps.append({
            "audiaT": np.ascontiguousarray(audia[b].T.astype(bf16)),
            "lipT": np.ascontiguousarray(lip[b].T.astype(bf16)),
            "WqkT": WqkT, "WlipT": WlipT, "WseT": WseT, "WprojN": WprojN,
            "bprojP": bprojP, "ident": ident, "bcHH": bcHH, "onesK": onesK,
        })
    return in_maps


def run(inputs, trace=False, **kw):
    from concourse.bass_utils import run_bass_kernel_spmd
    if "nc" not in _CACHED:
        _CACHED["nc"] = build_nc()
    in_maps = _marshal(**inputs)
    return run_bass_kernel_spmd(
        _CACHED["nc"], in_maps, core_ids=list(range(B)), trace=trace, **kw
    )


def kernel(audia, lip, W_qkv, W_lip, W_proj, b_proj, W_se):
    res = run(dict(audia=audia, lip=lip, W_qkv=W_qkv, W_lip=W_lip,
                   W_proj=W_proj, b_proj=b_proj, W_se=W_se))
    return np.stack([r["out"] for r in res.results], 0).astype(np.float32)



# revision 71
# speedup vs baseline: 1.0047x; 1.0047x over previous
"""Trainium2 Bass kernel for nn_AV_MiltiHeadAttention.

Strategy: data-parallel over B across 8 NeuronCores (1 batch element per core).
Per-core everything is kept in a "transposed" (feature-on-partitions) layout so
no on-chip transposes are needed:

  audiaT/lipT/W*T are marshalled on host (transpose + bf16 cast = input
  layout prep for the SPMD shards), all matmul contractions then have their
  contraction dim on SBUF partitions naturally.

  S.T[m,n] = sum_d kTz[d,m] qT[d,n]         (zero-padded per head: K=128, so
                                             every matmul shares one PE tiling
                                             mode -> no array-drain switches)
  E = exp(S.T * scale)                       (no max subtraction; |S*scale|<2)
  raw = [ones64|k_h].T @ E (parity-flipped) -> one matmul yields BOTH the
        unnormalized attn_qk.T rows AND the softmax denominator r broadcast
        across a 64-row block, fp32 in PSUM; a full-tile in-place DVE recip
        (base-0 constrained) gives 1/r, and the normalize tensor_tensor is
        fully partition-aligned.  No [1,N] lane-starved recip chains, no
        separate broadcast matmuls.
  y = attn_qk.T * lip_v.T ; expY = exp(y)
  1/s via ONE matmul against the bcHH block stationary ((d<64)==(j<64)):
  per-head head_dim sums of expY land already broadcast on all 128
  partitions (softmax over head_dim), fp32, then in-place recip.
  z.T = attn_qk.T * expY * (1/s)
  av_attn = 0.5 + 0.5*tanh(qs.T@qs / temp)   (SE layer; tanh lives in the
                                             same ACT table set as exp -> no
                                             mid-kernel ACT_TABLE_LOADs)
  Wc = W_proj.T @ av_attn ; row = b_proj @ av_attn   (proj/out fusion)
  out = z @ Wc + row                          (bias + cb<3 partials
                                             accumulate early, straddling
                                             the front(3)/back(3) stall
                                             windows; the partial re-enters
                                             the final psum via a K=128
                                             identity matmul so the tail is
                                             one evac + DMA per chunk)

All matmuls bf16 x bf16 -> fp32 PSUM. Softmax normalization in fp32.
"""

import os
import numpy as np
import ml_dtypes

DEBUG_DUMP = os.environ.get("KDBG", "0") == "1"

B, N, C = 8, 1024, 512
H, HD = 8, 64
CB = C // 128          # 4 chunks of the feature dim
MB = N // 128          # 8 chunks of the token dim
NH = N // 512          # 2 halves of the token dim (matmul free dim = 512)
SCALE = HD ** -0.5
TEMP = C ** 0.5

_CACHED = {}


def build_nc():
    import concourse.bass as bass
    import concourse.tile as tile
    import concourse.mybir as mybir
    from concourse import bacc
    from contextlib import ExitStack

    f32 = mybir.dt.float32
    bf16 = mybir.dt.bfloat16
    AF = mybir.ActivationFunctionType
    MUL = mybir.AluOpType.mult
    ADD = mybir.AluOpType.add

    nc = bacc.Bacc("TRN2", target_bir_lowering=False, debug=False, num_devices=B)

    d_audiaT = nc.dram_tensor("audiaT", [C, N], bf16, kind="ExternalInput")
    d_lipT = nc.dram_tensor("lipT", [C, N], bf16, kind="ExternalInput")
    d_WqkT = nc.dram_tensor("WqkT", [C, 2 * C], bf16, kind="ExternalInput")
    d_WlipT = nc.dram_tensor("WlipT", [C, C], bf16, kind="ExternalInput")
    d_WseT = nc.dram_tensor("WseT", [C, C], bf16, kind="ExternalInput")
    d_WprojN = nc.dram_tensor("WprojN", [C, C], bf16, kind="ExternalInput")
    d_bprojP = nc.dram_tensor("bprojP", [128, CB, HD + 1], bf16, kind="ExternalInput")
    d_ident = nc.dram_tensor("ident", [128, 128], bf16, kind="ExternalInput")
    d_bcHH = nc.dram_tensor("bcHH", [128, 128], bf16, kind="ExternalInput")
    d_onesK = nc.dram_tensor("onesK", [HD + 1, 128], bf16, kind="ExternalInput")
    d_out = nc.dram_tensor("out", [N, C], bf16, kind="ExternalOutput")
    dbg = {}
    if DEBUG_DUMP:
        for nm, shp, dt in [
            ("dbg_qT", [128, CB, N], bf16), ("dbg_kTz", [128, CB, 2, N], bf16),
            ("dbg_kaug", [128, MB, H * 2 * HD], bf16),
            ("dbg_lipv", [128, CB, N], bf16), ("dbg_qs", [128, MB, C], bf16),
            ("dbg_E0", [128, MB, 2 * N], bf16),
            ("dbg_rawsb0", [HD, N], bf16), ("dbg_attn0", [128, N], bf16),
            ("dbg_r00", [1, N], f32),
            ("dbg_y0", [128, N], bf16), ("dbg_expY0", [128, N], bf16),
            ("dbg_sblo0", [HD, N], f32), ("dbg_sbhi0", [HD, N], f32),
            ("dbg_zT", [128, CB, N], bf16), ("dbg_avattn", [128, CB, C], bf16),
            ("dbg_wc", [128, CB, C], bf16),
        ]:
            dbg[nm] = nc.dram_tensor(nm, shp, dt, kind="ExternalOutput")

    with tile.TileContext(nc) as tc, ExitStack() as ctx:
        persist = ctx.enter_context(tc.tile_pool(name="persist", bufs=1))
        # psA: 2x [128,1024] fp32 (2 banks each) for the S/qkT/partial/final
        # streams; psB: 4x [128,512] (1 bank each) -- fine-grained ring for
        # the raw/normalize chains so a held tile blocks less of the ring.
        psA = ctx.enter_context(tc.tile_pool(name="psA", bufs=2, space="PSUM"))
        psB = ctx.enter_context(tc.tile_pool(name="psB", bufs=4, space="PSUM"))

        # ---- persistent SBUF tensors ----
        qT = persist.tile([128, CB, N], bf16, tag="qT")        # q.T  [(h,d), n]
        # k.T zero-padded per head so the S matmuls run K=128 (no PE
        # tiling-mode switches): chunk 0 = [k_h_lo; 0], chunk 1 = [0; k_h_hi]
        # (measured: 64-row tiling loses ~18us to mode-switch drains because
        # the scheduler interleaves S with 128-mode matmuls, and the pairs
        # never co-stream -- the psA ring is ACT-gated.  Zeros via the idle
        # GpSimd engine, off the DVE.)
        kTz = persist.tile([128, CB, 2, N], bf16, tag="kTz")
        # k natural, one 128-col block per head: even heads [ones64 | k_h],
        # odd heads [k_h | ones64].  The raw matmul then emits BOTH the
        # unnormalized attn rows AND the softmax denominator r broadcast
        # across a 64-row block (in fp32, straight from the PE) -- and the
        # parity flip makes every downstream evac/recip/normalize-TT
        # partition-aligned with the packed raw2/attn_sb layout.
        k_aug = persist.tile([128, MB, H * 2 * HD], bf16, tag="k_aug")
        lip_vT = persist.tile([128, CB, N], bf16, tag="lip_vT")
        qs = persist.tile([128, MB, C], bf16, tag="qs")        # qs natural [n, c]
        WprojN_sb = persist.tile([128, CB, C], bf16, tag="WprojN_sb")
        Wc_sb = persist.tile([128, CB, C], bf16, tag="Wc_sb")
        bprojP_sb = persist.tile([128, CB, HD + 1], bf16, tag="bprojP_sb")
        ident_sb = persist.tile([128, 128], bf16, tag="ident_sb")
        # block stationary W[d,j] = ((d<64)==(j<64)): one matmul computes the
        # per-head head_dim sums of expY already BROADCAST across all 128
        # output partitions, in fp32 -- replaces the old sel-matmul ->
        # psum->sbuf s2 copies -> selC-broadcast-matmul chain entirely.
        bcHH_sb = persist.tile([128, 128], bf16, tag="bcHH_sb")
        zT = persist.tile([128, CB, N], bf16, tag="zT")
        av_attn = persist.tile([128, CB, C], bf16, tag="av_attn")
        # bias row for the output GEMM, as K=65 rhs (rows 1:64 zeroed so the
        # onesK zero-rows multiply clean values, never junk NaNs)
        row_bf = persist.tile([HD + 1, C], bf16, tag="row_bf")
        nc.gpsimd.memset(row_bf[:], 0.0)
        # (NOTE: a PE warm-up spin of dependency-free matmuls here was tried
        # to beat the HAM cold-clock and measured consistently ~25us SLOWER
        # overall -- do not reintroduce.)
        # onesK row 0 -> all cols (K=65 rounds to row-size 128: no PE
        # tiling-mode switch): folds the bias row into the partial-output
        # psum chains via one matmul.
        onesK = persist.tile([HD + 1, 128], bf16, tag="onesK")
        nc.gpsimd.dma_start(onesK[:], d_onesK[:])
        # kTz zero-padding memsets ride the early, otherwise-idle gpsimd
        # queue (the S(0) matmuls that read them start at ~17us)
        nc.gpsimd.memset(kTz[64:128, :, 0, :], 0.0)
        nc.gpsimd.memset(kTz[0:64, :, 1, :], 0.0)

        attn = ctx.enter_context(tc.tile_pool(name="attn", bufs=1))
        early_ctx = tc.tile_pool(name="early", bufs=1)
        early = early_ctx.__enter__()
        audiaT_sb = early.tile([128, CB, N], bf16, tag="audiaT_sb")
        lipT_sb = early.tile([128, CB, N], bf16, tag="lipT_sb")
        WqkT_sb = early.tile([128, CB, 2 * C], bf16, tag="WqkT_sb")
        WlipT_sb = early.tile([128, CB, C], bf16, tag="WlipT_sb")
        WseT_sb = early.tile([128, CB, C], bf16, tag="WseT_sb")

        # input DMAs ordered by first use and spread across engine DMA rings:
        # qkT needs WqkT(q half) [sync ring] + audiaT [scalar ring] first;
        # the k half [vector ring] feeds emit_kaug; lip/se/proj follow.
        d_WqkT_r = d_WqkT[:].rearrange("(cb p) c -> cb p c", p=128)
        d_audiaT_r = d_audiaT[:].rearrange("(cb p) n -> cb p n", p=128)
        # (NOTE: splitting the leading WqkT/audiaT chunks into smaller first
        # transfers was tried and measured ~4us SLOWER -- the extra ~650ns
        # descriptor submits delay the following chunks more than the
        # earlier first-semaphore gains.)
        for cb in range(CB):
            nc.sync.dma_start(WqkT_sb[:, cb, 0:C], d_WqkT_r[cb][:, 0:C])
            nc.scalar.dma_start(audiaT_sb[:, cb], d_audiaT_r[cb])
        for cb in range(CB):
            nc.sync.dma_start(WqkT_sb[:, cb, C:2 * C], d_WqkT_r[cb][:, C:2 * C])
        nc.scalar.dma_start(WlipT_sb[:], d_WlipT[:].rearrange("(cb p) c -> p cb c", p=128))
        nc.scalar.dma_start(lipT_sb[:], d_lipT[:].rearrange("(cb p) n -> p cb n", p=128))
        nc.sync.dma_start(bcHH_sb[:], d_bcHH[:])
        nc.sync.dma_start(WseT_sb[:], d_WseT[:].rearrange("(cb p) c -> p cb c", p=128))
        nc.scalar.dma_start(WprojN_sb[:], d_WprojN[:].rearrange("(cb p) c -> p cb c", p=128))
        nc.scalar.dma_start(bprojP_sb[:], d_bprojP[:])
        nc.gpsimd.dma_start(ident_sb[:], d_ident[:])



        # ---- P1 pieces (emitted interleaved with attention below) ----
        def emit_qkT():
            # qT / kTz: out [oc:128, n] ; lhsT = WqkT block, rhs = audiaT
            for dst, oc0 in ((qT, 0), (kTz, C)):
                for ocb in range(CB):
                    ps = psA.tile([128, N], f32, tag="psA")
                    for cb in range(CB):
                        for nh in range(NH):
                            nc.tensor.matmul(
                                ps[:, nh * 512:(nh + 1) * 512],
                                WqkT_sb[:, cb, oc0 + ocb * 128: oc0 + (ocb + 1) * 128],
                                audiaT_sb[:, cb, nh * 512:(nh + 1) * 512],
                                start=(cb == 0), stop=(cb == CB - 1),
                            )
                    if dst is qT:
                        nc.vector.tensor_copy(out=dst[:, ocb, :], in_=ps[:])
                    else:
                        nc.vector.tensor_copy(out=kTz[0:64, ocb, 0, :], in_=ps[0:64, :])
                        # DVE, not ACT: the E0/E1 exps own the ACT queue
                        # here and pace the whole stage
                        nc.vector.tensor_copy(out=kTz[64:128, ocb, 1, :],
                                              in_=ps[64:128, :])

        def emit_kaug():
            # k natural into the per-parity block slots; ones blocks memset
            # once up front (even heads: block 0 = ones; odd: block 1)
            kv = k_aug[:].rearrange(
                "p m (hp par two d) -> p m hp par two d", par=2, two=2, d=HD
            )
            nc.gpsimd.memset(kv[:, :, :, 0, 0, :], 1.0)
            nc.gpsimd.memset(kv[:, :, :, 1, 1, :], 1.0)
            for mb in range(MB):
                ps = psB.tile([128, 512], f32, tag="psB")
                for cb in range(CB):
                    nc.tensor.matmul(
                        ps[:],
                        audiaT_sb[:, cb, mb * 128:(mb + 1) * 128],
                        WqkT_sb[:, cb, C:2 * C],
                        start=(cb == 0), stop=(cb == CB - 1),
                    )
                psv = ps[:].rearrange("p (hp par d) -> p hp par d", par=2, d=HD)
                kvm = k_aug[:, mb].rearrange(
                    "p (hp par two d) -> p hp par two d", par=2, two=2, d=HD
                )
                nc.vector.tensor_copy(out=kvm[:, :, 0, 1, :], in_=psv[:, :, 0, :])
                nc.vector.tensor_copy(out=kvm[:, :, 1, 0, :], in_=psv[:, :, 1, :])

        def emit_lipv():
            for ocb in range(CB):
                pss = [psB.tile([128, 512], f32, tag="psB", name=f"lipv_ps{ocb}_{i}")
                       for i in range(NH)]
                for cb in range(CB):
                    for nh in range(NH):
                        nc.tensor.matmul(
                            pss[nh][:],
                            WlipT_sb[:, cb, ocb * 128:(ocb + 1) * 128],
                            lipT_sb[:, cb, nh * 512:(nh + 1) * 512],
                            start=(cb == 0), stop=(cb == CB - 1),
                        )
                for nh in range(NH):
                    nc.vector.tensor_copy(
                        out=lip_vT[:, ocb, nh * 512:(nh + 1) * 512], in_=pss[nh][:]
                    )

        def emit_qs():
            # qs natural [n, c] (needed on both sides of the SE bmm)
            for nb in range(MB):
                ps = psB.tile([128, 512], f32, tag="psB")
                for cb in range(CB):
                    nc.tensor.matmul(
                        ps[:],
                        audiaT_sb[:, cb, nb * 128:(nb + 1) * 128],
                        WseT_sb[:, cb, :],
                        start=(cb == 0), stop=(cb == CB - 1),
                    )
                nc.vector.tensor_copy(out=qs[:, nb, :], in_=ps[:])

        # ---- attention + SE ----
        def emit_se():
            # x[c,d] = sum_n qs[n,c] qs[n,d];  av_attn = sigmoid(2x/temp)
            # computed as 0.5 + 0.5*tanh(x/temp): tanh lives in the same ACT
            # table set as exp, so no mid-kernel ACT_TABLE_LOAD thrash.
            # (NOTE: evacuating this psum via a DVE cast to SBUF staging --
            # to free the psB tile before the E3-queued tanh reads it --
            # measured neutral-to-slower: the extra copies lengthen the same
            # late DVE queue that front(3)'s normalize chain rides.)
            for cb in range(CB):
                ps = psB.tile([128, 512], f32, tag="psB")
                for nb in range(MB):
                    nc.tensor.matmul(
                        ps[:],
                        qs[:, nb, cb * 128:(cb + 1) * 128],
                        qs[:, nb, :],
                        start=(nb == 0), stop=(nb == MB - 1),
                    )
                th = attn.tile([128, C], bf16, tag="th", bufs=2, name=f"th{cb}")
                nc.scalar.activation(th[:], ps[:], AF.Tanh, scale=1.0 / TEMP)
                nc.vector.tensor_scalar(av_attn[:, cb, :], th[:], 0.5, 0.5, MUL, ADD)

        def emit_wc():
            # Wc = W_proj.T @ av_attn  (fold the proj GEMM into the output GEMM)
            for ccb in range(CB):
                ps = psA.tile([128, N], f32, tag="psA")
                for eb in range(CB):
                    nc.tensor.matmul(
                        ps[:, 0:512],
                        WprojN_sb[:, eb, ccb * 128:(ccb + 1) * 128],
                        av_attn[:, eb, :],
                        start=(eb == 0), stop=(eb == CB - 1),
                    )
                # DVE, not ACT: the E3 exps own the ACT queue here
                nc.vector.tensor_copy(out=Wc_sb[:, ccb, :], in_=ps[:, 0:512])
            # row = b_proj @ av_attn (M=65 zero-padded stationary: no tiling
            # switch); prefolded into the partial-output psum chains via a
            # K=65 onesK matmul.
            rp = psB.tile([128, 512], f32, tag="psB")
            for cb in range(CB):
                nc.tensor.matmul(
                    rp[0:HD + 1, :],
                    bprojP_sb[:, cb, :],
                    av_attn[:, cb, :],
                    start=(cb == 0), stop=(cb == CB - 1),
                )
            nc.vector.tensor_copy(out=row_bf[0:1, :], in_=rp[0:1, :])
            if DEBUG_DUMP:
                nc.sync.dma_start(dbg["dbg_wc"][:], Wc_sb[:])

        def emit_S(p):
            # E = exp(S.T * scale) for heads (2p, 2p+1)
            E = attn.tile([128, MB, 2 * N], bf16, tag="E", bufs=2, name=f"E{p}")
            for mb in range(MB):
                for hh in range(2):
                    ps = psA.tile([128, N], f32, tag="psA")
                    for nh in range(NH):
                        nc.tensor.matmul(
                            ps[:, nh * 512:(nh + 1) * 512],
                            kTz[:, p, hh, mb * 128:(mb + 1) * 128],
                            qT[:, p, nh * 512:(nh + 1) * 512],
                            start=True, stop=True,
                        )
                    nc.scalar.activation(
                        E[:, mb, hh * N:(hh + 1) * N], ps[:], AF.Exp, scale=SCALE
                    )
            return E

        def emit_qk_front(p, E):
            # raw = [k_h|1].T @ E_h for BOTH heads back-to-back (keeps the PE
            # dense so HAM stays at full clock), then the first-softmax
            # normalize chain through y/expY.  The s-path + z live in
            # emit_qk_back so PE filler work can be emitted between them
            # (the PE queue is in-order: a stalled s_ps matmul would block
            # every filler emitted after it).
            attn_sb = attn.tile([128, N], bf16, tag="attn_sb", bufs=2, name=f"attn_sb{p}")
            raw2 = attn.tile([128, N], bf16, tag="raw_sb", bufs=2, name=f"raw2_{p}")
            rawps = {}
            for hh in range(2):
                h = 2 * p + hh
                raws = [psB.tile([128, 512], f32, tag="psB", name=f"raw{p}_{hh}_{i}")
                        for i in range(NH)]
                rawps[hh] = raws
                for mb in range(MB):
                    for nh in range(NH):
                        nc.tensor.matmul(
                            raws[nh][:],
                            k_aug[:, mb, h * 128:(h + 1) * 128],
                            E[:, mb, hh * N + nh * 512: hh * N + (nh + 1) * 512],
                            start=(mb == 0), stop=(mb == MB - 1),
                        )
            # even head: psum rows 0:64 = r broadcast, rows 64:128 = unnorm
            # attn; odd head mirrored.  Normalize runs nh-outer so each
            # column half flows evac -> recip -> TT -> y -> expY as one
            # short chain (expY gates the s-path matmuls downstream).
            # Evacs: the attn rows move to the packed raw2 via partition-
            # shifted DVE copies; the full-tile in-place recip keeps the op
            # at partition base 0 (the DVE recip silently no-ops at base 64)
            # -- the attn rows it clobbers are already in raw2.  All evacs
            # stay off ACT: the exp streams own that queue and pace every
            # stage (an ACT-routed p==2 evac measured slower).
            y = attn.tile([128, N], bf16, tag="y", bufs=2, name=f"y{p}")
            expY = attn.tile([128, N], bf16, tag="expY", bufs=2, name=f"expY{p}")
            for nh in range(NH):
                sl = slice(nh * 512, (nh + 1) * 512)
                for hh in range(2):
                    a0 = 64 if hh == 0 else 0     # attn rows base in psum
                    nc.vector.tensor_copy(
                        out=raw2[hh * 64:hh * 64 + 64, sl],
                        in_=rawps[hh][nh][a0:a0 + 64, :],
                    )
                for hh in range(2):
                    nc.vector.reciprocal_approx_fast(
                        out=rawps[hh][nh][:], in_=rawps[hh][nh][:]
                    )
                for hh in range(2):
                    r0 = 0 if hh == 0 else 64
                    rows = slice(hh * 64, hh * 64 + 64)
                    nc.vector.tensor_tensor(
                        attn_sb[rows, sl], raw2[rows, sl],
                        rawps[hh][nh][r0:r0 + 64, :], MUL,
                    )
                # y gates expY -> the whole s-chain: keep it on the fast DVE
                # (a GpSimd TT takes 1.15us vs 0.33us and measured as the
                # largest late-phase PE gap when p==2 rode GpSimd).
                nc.vector.tensor_tensor(y[:, sl], attn_sb[:, sl],
                                        lip_vT[:, p, sl], MUL)
                nc.scalar.activation(expY[:, sl], y[:, sl], AF.Exp)
            if DEBUG_DUMP and p == 0:
                nc.sync.dma_start(dbg["dbg_rawsb0"][:], raw2[0:HD, :])
                nc.sync.dma_start(dbg["dbg_attn0"][:], attn_sb[:])
                nc.sync.dma_start(dbg["dbg_y0"][:], y[:])
                nc.sync.dma_start(dbg["dbg_expY0"][:], expY[:])
            return attn_sb, expY

        def emit_qk_back(p, attn_sb, expY):
            # ONE matmul against the bcHH block stationary computes the
            # per-head head_dim sums of expY already broadcast across all
            # 128 output partitions (rows 0:64 = s_lo, rows 64:128 = s_hi,
            # in fp32 -- no bf16 round-trip of s); an in-place full-width
            # reciprocal then yields 1/s.  The path runs at nh-half
            # granularity so zT's first half lands early - the output
            # finals for nb 0..3 only need columns 0:512.
            # For p==2 park the psums on psA - they hold their buffers to
            # the end of the chain and would starve the psB ring the se/wc
            # fillers need.  (p==3 must stay on psB: psA holds the output
            # partials.)
            if p == 2:
                sbt = psA.tile([128, N], f32, tag="psA")
                sbps = [sbt[:, 0:512], sbt[:, 512:1024]]
            else:
                sbps = [psB.tile([128, 512], f32, tag="psB", name=f"sb{p}_{i}")[:]
                        for i in range(NH)]
            u = attn.tile([128, N], bf16, tag="y", bufs=2, name=f"u{p}")
            for nh in range(NH):
                sl = slice(nh * 512, (nh + 1) * 512)
                nc.tensor.matmul(sbps[nh], bcHH_sb[:], expY[:, sl],
                                 start=True, stop=True)
                nc.vector.reciprocal_approx_fast(out=sbps[nh], in_=sbps[nh])
                # u is off the latency chain for p<2 (zT isn't needed until
                # the partials) -> GpSimd; p>=2 stays DVE (partials/finals
                # wait on zT 2/3, and a GpSimd TT costs 1.15us).
                if p <= 1:
                    nc.gpsimd.tensor_tensor(u[:, sl], attn_sb[:, sl], expY[:, sl], MUL)
                else:
                    nc.vector.tensor_tensor(u[:, sl], attn_sb[:, sl], expY[:, sl], MUL)
                nc.vector.tensor_tensor(zT[:, p, sl], u[:, sl], sbps[nh], MUL)

        # 2-deep software pipeline: S(p+1) overlaps qk(p); the independent
        # projection/SE matmuls are spread through the ACT-bound S windows
        # as PE filler.  se/wc fill qk(2)'s normalize window; the bias +
        # cb=0..2 partial output accumulation (evacuated to SBUF bf16 by the
        # idle ACT engine) fills qk(3)'s; only the cb=3 matmul + one add +
        # DMA remain after zT(3) lands.
        emit_qkT()
        E0 = emit_S(0)
        emit_kaug()
        E1 = emit_S(1)
        emit_lipv()
        fb0 = emit_qk_front(0, E0)
        if DEBUG_DUMP:
            nc.sync.dma_start(dbg["dbg_E0"][:], E0[:])
        E2 = emit_S(2)
        emit_qk_back(0, *fb0)
        emit_qs()
        early_ctx.__exit__(None, None, None)
        fb1 = emit_qk_front(1, E1)
        E3 = emit_S(3)
        emit_qk_back(1, *fb1)
        fb2 = emit_qk_front(2, E2)
        # NOTE: emitting se two stages earlier (to pull its tanh ahead of
        # the E3 exps) measured ~30us SLOWER -- its psB tiles hostage the
        # mid-stage ring.  Keep se/wc here.
        emit_se()
        emit_wc()
        emit_qk_back(2, *fb2)
        # bias + cb=0..2 partial output accumulation for all 8 nb chunks;
        # two nb chunks pack into each [128,1024] psA tile, evacuated to
        # partial_sb so the psA ring keeps cycling.
        late = ctx.enter_context(tc.tile_pool(name="late", bufs=1))
        partial_sb = late.tile([128, MB, C], bf16, tag="partial_sb")

        def emit_partials(ilo, ihi):
            for i in range(ilo, ihi):
                ps = psA.tile([128, N], f32, tag="psA")
                for half in range(2):
                    nb = 2 * i + half
                    nc.tensor.matmul(
                        ps[:, half * 512:(half + 1) * 512],
                        onesK[:], row_bf[:],
                        start=True, stop=False,
                    )
                    for cb in range(CB - 1):
                        nc.tensor.matmul(
                            ps[:, half * 512:(half + 1) * 512],
                            zT[:, cb, nb * 128:(nb + 1) * 128],
                            Wc_sb[:, cb, :],
                            start=False, stop=(cb == CB - 2),
                        )
                # all evacs on DVE: the E3 exps own the ACT queue in this
                # window and pace the stage
                for half in range(2):
                    nc.vector.tensor_copy(
                        out=partial_sb[:, 2 * i + half, :],
                        in_=ps[:, half * 512:(half + 1) * 512],
                    )

        # partials straddle BOTH stall windows: ~2 i-slots of ready matmul
        # work sit in the PE queue while front(3)'s DVE normalize chain
        # runs (else a blocked bcHH(3) at the queue head costs ~4us), and
        # one more slot fills back(3)'s s-chain window before the finals.
        emit_partials(0, 1)
        fb3 = emit_qk_front(3, E3)
        emit_partials(1, 3)
        emit_qk_back(3, *fb3)
        # nb 6,7 (the tail-critical chunks) accumulate bias + cb0..2
        # directly in psB -- no partial_sb round-trip, no identity matmul:
        # the finals just add cb3 and evacuate.  (The open accumulation
        # group survives interleaved matmuls to other banks: has_written
        # bits are per-element per-bank.)
        direct = [psB.tile([128, 512], f32, tag="psB", name=f"direct{nb}")
                  for nb in (6, 7)]
        for j, nb in enumerate((6, 7)):
            nc.tensor.matmul(direct[j][:], onesK[:], row_bf[:],
                             start=True, stop=False)
            for cb in range(CB - 1):
                nc.tensor.matmul(
                    direct[j][:],
                    zT[:, cb, nb * 128:(nb + 1) * 128],
                    Wc_sb[:, cb, :],
                    start=False, stop=False,
                )
        if DEBUG_DUMP:
            nc.sync.dma_start(dbg["dbg_qT"][:], qT[:])
            nc.sync.dma_start(dbg["dbg_kTz"][:], kTz[:])
            nc.sync.dma_start(dbg["dbg_kaug"][:], k_aug[:])
            nc.sync.dma_start(dbg["dbg_lipv"][:], lip_vT[:])
            nc.sync.dma_start(dbg["dbg_qs"][:], qs[:])
            nc.sync.dma_start(dbg["dbg_zT"][:], zT[:])
            nc.sync.dma_start(dbg["dbg_avattn"][:], av_attn[:])

        # ---- output finals: out[nb] = partial_sb[nb] + z[:,3] @ Wc[3] ----
        # the partial re-enters through the PE (K=128 identity matmul
        # accumulating into the same psum), so the tail needs only one
        # psum->sbuf evacuation per [128,1024] (alternating ACT/DVE) + DMA.
        d_out_r = d_out[:].rearrange("(nb p) c -> nb p c", p=128)
        with tc.tile_pool(name="outp", bufs=3) as outp:
            for i in range(3):
                ps = psA.tile([128, N], f32, tag="psA")
                for half in range(2):
                    nb = 2 * i + half
                    sl = slice(half * 512, (half + 1) * 512)
                    nc.tensor.matmul(
                        ps[:, sl],
                        zT[:, CB - 1, nb * 128:(nb + 1) * 128],
                        Wc_sb[:, CB - 1, :],
                        start=True, stop=False,
                    )
                    nc.tensor.matmul(
                        ps[:, sl], ident_sb[:], partial_sb[:, 2 * i + half, :],
                        start=False, stop=True,
                    )
                # one whole-tile evac per i, alternating engines (a per-half
                # ACT||DVE split was tried and measured ~4us slower: the ACT
                # halves collide with the expY(3)/o6 tail stream)
                o2 = outp.tile([128, 2, C], bf16, tag="o_sb")
                if i % 2 == 0:
                    nc.scalar.activation(o2[:], ps[:], AF.Copy)
                else:
                    nc.vector.tensor_copy(out=o2[:], in_=ps[:])
                for half in range(2):
                    nb = 2 * i + half
                    # drain the stores over the scalar+sync DMA rings only:
                    # a gpsimd-ring store here costs a ~2.3us gpsimd pipe
                    # DRAIN in the postamble, after everything else is done
                    if nb % 2 == 0:
                        nc.scalar.dma_start(d_out_r[nb], o2[:, half])
                    else:
                        nc.sync.dma_start(d_out_r[nb], o2[:, half])
            # nb 6,7: close the direct psB accumulation with the cb3 matmul,
            # one evac each (ACT/DVE split), DMA on separate rings
            o6 = outp.tile([128, C], bf16, tag="o_sb", name="o6")
            o7 = outp.tile([128, C], bf16, tag="o_sb", name="o7")
            for j, nb in enumerate((6, 7)):
                nc.tensor.matmul(
                    direct[j][:],
                    zT[:, CB - 1, nb * 128:(nb + 1) * 128],
                    Wc_sb[:, CB - 1, :],
                    start=False, stop=True,
                )
            nc.scalar.activation(o6[:], direct[0][:], AF.Copy)
            nc.vector.tensor_copy(out=o7[:], in_=direct[1][:])
            nc.scalar.dma_start(d_out_r[6], o6[:])
            nc.sync.dma_start(d_out_r[7], o7[:])

    nc.compile()
    return nc


def _marshal(audia, lip, W_qkv, W_lip, W_proj, b_proj, W_se):
    bf16 = ml_dtypes.bfloat16
    WqkT = np.ascontiguousarray(W_qkv[:2 * C].T.astype(bf16))
    WlipT = np.ascontiguousarray(W_lip.T.astype(bf16))
    WseT = np.ascontiguousarray(W_se.T.astype(bf16))
    WprojN = np.ascontiguousarray(W_proj.astype(bf16))
    bprojP = np.zeros((128, CB, HD + 1), bf16)
    bprojP[:, :, 0] = np.asarray(b_proj, np.float32).reshape(CB, 128).T.astype(bf16)
    ident = np.eye(128, dtype=np.float32).astype(bf16)
    bcHH = np.zeros((128, 128), bf16)
    bcHH[0:64, 0:64] = 1
    bcHH[64:128, 64:128] = 1
    onesK = np.zeros((HD + 1, 128), bf16)
    onesK[0, :] = 1
    in_maps = []
    for b in range(B):
        in_maps.append({
            "audiaT": np.ascontiguousarray(audia[b].T.astype(bf16)),
            "lipT": np.ascontiguousarray(lip[b].T.astype(bf16)),
            "WqkT": WqkT, "WlipT": WlipT, "WseT": WseT, "WprojN": WprojN,
            "bprojP": bprojP, "ident": ident, "bcHH": bcHH, "onesK": onesK,
        })
    return in_maps


def run(inputs, trace=False, **kw):
    from concourse.bass_utils import run_bass_kernel_spmd
    if "nc" not in _CACHED:
        _CACHED["nc"] = build_nc()
    in_maps = _marshal(**inputs)
    return run_bass_kernel_spmd(
        _CACHED["nc"], in_maps, core_ids=list(range(B)), trace=trace, **kw
    )


def kernel(audia, lip, W_qkv, W_lip, W_proj, b_proj, W_se):
    res = run(dict(audia=audia, lip=lip, W_qkv=W_qkv, W_lip=W_lip,
                   W_proj=W_proj, b_proj=b_proj, W_se=W_se))
    return np.stack([r["out"] for r in res.results], 0).astype(np.float32)

